# revision 1
# baseline (speedup 1.0000x reference)
"""Trainium2 Bass kernel for AdvancedAudioStegEncoder (B=4, L=4096, 8 cores).

Sharding: hybrid batch x sequence. Core c handles batch b=c//2, sequence
half h=c%2 (owns positions [h*2048, h*2048+2048)). Each core computes all
layers for its shard with a +-32 position margin (recompute instead of halo
exchange). Cross-core communication:
  - 3x AllReduce (8 cores) of per-channel BatchNorm batch statistics
  - 2x AllGather (pairs) of attention K / V^T for the full 4096-key axis
Attention is computed in scores-transposed layout [keys->partitions,
queries->free] so softmax probs feed the AV matmul directly with no
transposes; the softmax normalizer comes from an extra ones-column (attn1)
or a ones-lhsT matmul (attn2); exp needs no max-subtraction (scores are
bounded ~+-22, verified against the reference).
"""
import sys
import numpy as np

sys.path.insert(0, "/opt/trn_rl_repo")

import ml_dtypes

import concourse.bass as bass
import concourse.bacc as bacc
import concourse.tile as tile
import concourse.mybir as mybir
from concourse.bass_utils import run_bass_kernel_spmd
from concourse.bass import broadcast_tensor_aps

BF16 = mybir.dt.bfloat16
F32 = mybir.dt.float32
AF = mybir.ActivationFunctionType
ALU = mybir.AluOpType

B, L = 4, 4096
NCORES = 8
OWN = 2048          # owned positions per core
MG = 32             # margin each side
W = OWN + 2 * MG    # 2112 working width
PD = 8              # zero padding columns each side of conv-input tiles
WP = W + 2 * PD     # 2128
INM = 40            # input margin (conv1 needs +-39)
WIN = OWN + 2 * INM  # 2128
EPS = 1e-5
NSTAT = float(B * L)
STRENGTH = 0.01

CHUNKS = [(0, 512), (512, 512), (1024, 512), (1536, 512), (2048, 64)]
# owned sub-slices of each chunk (tile coords) for BN statistics
STAT_SL = [(32, 480), (512, 512), (1024, 512), (1536, 512), (2048, 32)]
# conv5 / output chunks cover the owned region only
OUT_CHUNKS = [(32, 512), (544, 512), (1056, 512), (1568, 512)]

JT = 32   # total key tiles (4096 / 128)
JL = 16   # local key tiles

LAST_RESULTS = None  # stashed BassKernelResults for test.py


def _bf(x):
    return np.ascontiguousarray(x).astype(ml_dtypes.bfloat16)


def _f32(x):
    return np.ascontiguousarray(x).astype(np.float32)


def build_graph(gamma1: float, gamma2: float, c5b_f: float):
    nc = bacc.Bacc("TRN2", target_bir_lowering=False, debug=False,
                   num_devices=NCORES)

    def din(name, shape, dt=BF16):
        return nc.dram_tensor(name, shape, dt, kind="ExternalInput")

    # per-core inputs
    x0_d = din("x0", [2, WIN], F32)
    aud_d = din("aud", [1, OWN], F32)
    mask_d = din("mask", [1, W], F32)
    m01_d = din("m01", [128, 2], F32)   # col0 = (1-h), col1 = h  select weights
    # weights (same on all cores)
    w1c_d = din("w1c", [30, 64], F32)
    wq1_d = din("wq1", [64, 8]); wk1_d = din("wk1", [64, 8])
    wv1_d = din("wv1", [64, 64])
    w2p_d = din("w2p", [128, 8, 128])
    wq2_d = din("wq2", [128, 16]); wk2_d = din("wk2", [128, 16])
    wv2_d = din("wv2", [128, 128])
    w3t_d = din("w3t", [128, 15, 64])
    w4p_d = din("w4p", [128, 8, 32])
    w5g_d = din("w5g", [128, 4, 1])
    bn1g_d = din("bn1g", [64, 1], F32); bn1b_d = din("bn1b", [64, 1], F32)
    bn2g_d = din("bn2g", [128, 1], F32); bn2b_d = din("bn2b", [128, 1], F32)
    bn3g_d = din("bn3g", [64, 1], F32); bn3b_d = din("bn3b", [64, 1], F32)
    c4b_d = din("c4b", [32, 1], F32)

    out_d = nc.dram_tensor("out", [1, OWN], F32, kind="ExternalOutput")

    # collective bounce buffers (pair AG outputs must NOT be Shared)
    AG1N = 2048 * 65 + 8 * 2048
    AG2N = 2048 * 128 + 16 * 2048
    ag1_in = nc.dram_tensor("ag1_in", [AG1N], BF16, kind="Internal")
    ag1_out = nc.dram_tensor("ag1_out", [2, AG1N], BF16, kind="Internal")
    ag2_in = nc.dram_tensor("ag2_in", [AG2N], BF16, kind="Internal")
    ag2_out = nc.dram_tensor("ag2_out", [2, AG2N], BF16, kind="Internal")
    ar_in = [nc.dram_tensor(f"ar{i}_in", [128, 2], F32, kind="Internal")
             for i in range(3)]
    ar_out = [nc.dram_tensor(f"ar{i}_out", [8, 128, 2], F32, kind="Internal",
                             addr_space="Shared") for i in range(3)]

    PAIRS = [[0, 1], [2, 3], [4, 5], [6, 7]]
    ALL8 = [list(range(8))]

    with tile.TileContext(nc) as tc:
        with tc.tile_pool(name="const", bufs=1) as cp, \
             tc.tile_pool(name="act", bufs=1) as ap_, \
             tc.tile_pool(name="probs", bufs=1) as pp, \
             tc.tile_pool(name="small", bufs=1) as sp, \
             tc.tile_pool(name="psA", bufs=4, space="PSUM") as psA, \
             tc.tile_pool(name="psB", bufs=1, space="PSUM") as psB, \
             tc.tile_pool(name="psC", bufs=3, space="PSUM") as psC:

            # ---------- load constants (spread across DMA queues) ----------
            _eng = [nc.sync, nc.scalar]
            _ei = [0]

            def load(dram, shape, dt=BF16):
                t = cp.tile(shape, dt, tag=f"c_{dram.name}")
                _eng[_ei[0] % len(_eng)].dma_start(t[:], dram.ap())
                _ei[0] += 1
                return t

            # critical-path loads first: conv1 inputs, then layer-1 weights
            w1c = load(w1c_d, [30, 64], F32)
            mask = load(mask_d, [1, W], F32)
            m01 = load(m01_d, [128, 2], F32)
            x0d = ap_.tile([30, W], F32, tag="x0d")
            for t in range(15):
                nc.sync.dma_start(x0d[2 * t:2 * t + 2, :],
                                  x0_d[:, t + 1:t + 1 + W])
            bn1g = load(bn1g_d, [64, 1], F32); bn1b = load(bn1b_d, [64, 1], F32)
            wq1 = load(wq1_d, [64, 8]); wk1 = load(wk1_d, [64, 8])
            wv1 = load(wv1_d, [64, 64])
            ones = cp.tile([128, 1], BF16, tag="c_ones")
            nc.vector.memset(ones[:], 1.0)
            mask_b = cp.tile([128, W], F32, tag="c_maskb")
            nc.gpsimd.partition_broadcast(mask_b[:], mask[:])
            # later-layer weights load behind layer-1 compute
            w2p = load(w2p_d, [128, 8, 128])
            wq2 = load(wq2_d, [128, 16]); wk2 = load(wk2_d, [128, 16])
            wv2 = load(wv2_d, [128, 128])
            w3t = load(w3t_d, [128, 15, 64])
            w4p = load(w4p_d, [128, 8, 32])
            w5g = load(w5g_d, [128, 4, 1])
            bn2g = load(bn2g_d, [128, 1], F32); bn2b = load(bn2b_d, [128, 1], F32)
            bn3g = load(bn3g_d, [64, 1], F32); bn3b = load(bn3b_d, [64, 1], F32)
            c4b = load(c4b_d, [32, 1], F32)
            aud = load(aud_d, [1, OWN], F32)

            # ---------- conv1 (2 -> 64, via im2col K=30) ----------

            st1 = sp.tile([64, 16], F32, tag="st1")   # cols 0-4 sums, 5-9 sumsq
            sq1 = sp.tile([64, 600], F32, tag="sq")  # TTR elementwise-out scratch
            y1 = ap_.tile([64, W], BF16, tag="y")
            for ci, (cs, cw) in enumerate(CHUNKS):
                ps = psA.tile([64, 512], F32, tag="ps")
                nc.tensor.matmul(ps[:, 0:cw], w1c[:], x0d[:, cs:cs + cw],
                                 start=True, stop=True)
                a, wd = STAT_SL[ci]
                sl = ps[0:64, a - cs:a - cs + wd]
                nc.vector.tensor_reduce(st1[:, ci:ci + 1], sl,
                                        axis=mybir.AxisListType.X, op=ALU.add)
                nc.scalar.activation(sq1[:, 0:wd], sl, AF.Square,
                                     accum_out=st1[:, 5 + ci:6 + ci])
                nc.vector.tensor_copy(y1[:, cs:cs + cw], ps[:, 0:cw])

            # pack stats + all-reduce
            stats1 = sp.tile([64, 2], F32, tag="stats1")
            nc.vector.tensor_reduce(stats1[:, 0:1], st1[:, 0:5],
                                    axis=mybir.AxisListType.X, op=ALU.add)
            nc.vector.tensor_reduce(stats1[:, 1:2], st1[:, 5:10],
                                    axis=mybir.AxisListType.X, op=ALU.add)
            nc.sync.dma_start(ar_in[0][0:64, :], stats1[:])
            nc.gpsimd.collective_compute(
                "AllGather", ALU.bypass, replica_groups=ALL8,
                ins=[ar_in[0].ap().opt()], outs=[ar_out[0].ap().opt()])

            def bn_scale_shift(ar_dram, g, b, C, tag):
                """returns (scale, shift) [C,1] f32 from all-reduced [C,2] stats"""
                s = sp.tile([C, 8], F32, tag=tag)
                s8 = sp.tile([C, 8, 2], F32, tag=tag + "g")
                nc.sync.dma_start(
                    s8[:], ar_dram.ap()[:, 0:C, :].rearrange("b p c -> p b c"))
                nc.vector.tensor_reduce(s[:, 0:2], s8[:].rearrange("p b c -> p c b"),
                                        axis=mybir.AxisListType.X, op=ALU.add)
                # mean = s0/N ; msq = s1/N
                nc.vector.tensor_scalar_mul(s[:, 2:3], s[:, 0:1], 1.0 / NSTAT)
                nc.vector.tensor_scalar_mul(s[:, 3:4], s[:, 1:2], 1.0 / NSTAT)
                # var = msq - mean^2 + eps
                nc.vector.tensor_mul(s[:, 4:5], s[:, 2:3], s[:, 2:3])
                nc.vector.tensor_sub(s[:, 4:5], s[:, 3:4], s[:, 4:5])
                nc.vector.tensor_scalar_add(s[:, 4:5], s[:, 4:5], EPS)
                # rstd = sqrt(1/var) ; scale = g*rstd ; shift = b - mean*scale
                nc.vector.reciprocal(s[:, 5:6], s[:, 4:5])
                nc.scalar.sqrt(s[:, 6:7], s[:, 5:6])
                nc.vector.tensor_mul(s[:, 6:7], s[:, 6:7], g[:])
                nc.vector.tensor_mul(s[:, 7:8], s[:, 2:3], s[:, 6:7])
                nc.vector.tensor_sub(s[:, 7:8], b[:], s[:, 7:8])
                return s

            def bn_apply_mask(y, x_out, ss, C, xo=0, po=0):
                """x_out[po:po+C, xo:xo+W] = relu(y*scale+shift) * mask"""
                nc.vector.tensor_scalar(
                    x_out[po:po + C, xo:xo + W], y[0:C, :],
                    scalar1=ss[:, 6:7], scalar2=ss[:, 7:8],
                    op0=ALU.mult, op1=ALU.add)
                nc.vector.tensor_scalar_max(
                    x_out[po:po + C, xo:xo + W], x_out[po:po + C, xo:xo + W], 0.0)
                nc.vector.tensor_mul(x_out[po:po + C, xo:xo + W],
                                     x_out[po:po + C, xo:xo + W],
                                     mask_b[0:C, :])

            ss1 = bn_scale_shift(ar_out[0], bn1g, bn1b, 64, "ss1")
            x1 = ap_.tile([64, W], BF16, tag="x1")
            bn_apply_mask(y1, x1, ss1, 64)

            # ---------- attention 1 (C=64, d=8) ----------
            def attention(x, C, d, wq, wk, wv, gamma, ag_in_d, ag_out_d,
                          vcols, probs, xa_out, xa_po, xa_xo, ltag):
                """x: [C, W] bf16 masked. Writes attn output (masked) into
                xa_out[xa_po:xa_po+C, xa_xo:xa_xo+W]. vcols = C+1 (attn1
                ones-column) or C (attn2, Z via ones-lhsT matmul)."""
                with_ones = vcols == C + 1
                # projections: q [d, W], k_own [d, OWN], vT tiles [128, JL, vcols]
                q = sp.tile([d, W], BF16, tag="at_q")
                kown = sp.tile([d, OWN], BF16, tag="at_kown")
                for ci in range(4):
                    cs = MG + 512 * ci
                    ps = psB.tile([d, 512], F32, tag="psb")
                    nc.tensor.matmul(ps[:], wk[:], x[:, cs:cs + 512],
                                     start=True, stop=True)
                    nc.vector.tensor_copy(kown[:, 512 * ci:512 * ci + 512], ps[:])
                vT = sp.tile([128, JL, vcols], BF16, tag="at_vT")
                if with_ones:
                    nc.vector.memset(vT[:, :, C:C + 1], 1.0)
                for j in range(JL):
                    ps = psB.tile([128, vcols], F32, tag="psb")
                    nc.tensor.matmul(ps[:, 0:C], x[:, MG + 128 * j:MG + 128 * j + 128],
                                     wv[:], start=True, stop=True)
                    nc.vector.tensor_copy(vT[:, j, 0:C], ps[:, 0:C])
                    nc.sync.dma_start(
                        ag_in_d.ap()[128 * vcols * j: 128 * vcols * (j + 1)]
                        .rearrange("(p c) -> p c", p=128),
                        vT[:, j, :])
                nc.sync.dma_start(
                    ag_in_d.ap()[2048 * vcols:].rearrange("(d n) -> d n", d=d),
                    kown[:])
                nc.gpsimd.collective_compute(
                    "AllGather", ALU.bypass, replica_groups=PAIRS,
                    ins=[ag_in_d.ap().opt()], outs=[ag_out_d.ap().opt()])
                for ci, (cs, cw) in enumerate(CHUNKS):
                    ps = psB.tile([d, 512], F32, tag="psb")
                    nc.tensor.matmul(ps[:, 0:cw], wq[:], x[:, cs:cs + cw],
                                     start=True, stop=True)
                    nc.vector.tensor_copy(q[:, cs:cs + cw], ps[:, 0:cw])
                # remote kv: combine the two gathered blocks with select weights
                vTr = sp.tile([128, JL, vcols], BF16, tag="at_vTr")
                krem = sp.tile([d, OWN], BF16, tag="at_krem")
                vb = sp.tile([128, 2, JL, vcols], BF16, tag="at_vb")
                kb = sp.tile([d, 2, OWN], BF16, tag="at_kb")
                for blk in range(2):
                    nc.sync.dma_start(
                        vb[:, blk, :, :],
                        ag_out_d[blk, 0:2048 * vcols]
                        .rearrange("(j p c) -> p j c", p=128, c=vcols))
                    nc.sync.dma_start(
                        kb[:, blk, :],
                        ag_out_d[blk, 2048 * vcols:]
                        .rearrange("(d n) -> d n", d=d))
                # remote = b0*(1-h) + b1*h
                nc.vector.tensor_scalar_mul(vTr[:], vb[:, 0, :, :], m01[:, 0:1])
                nc.vector.scalar_tensor_tensor(
                    out=vTr[:], in0=vb[:, 1, :, :], scalar=m01[:, 1:2],
                    in1=vTr[:], op0=ALU.mult, op1=ALU.add)
                nc.vector.tensor_scalar_mul(krem[:], kb[:, 0, :], m01[0:d, 0:1])
                nc.vector.scalar_tensor_tensor(
                    out=krem[:], in0=kb[:, 1, :], scalar=m01[0:d, 1:2],
                    in1=krem[:], op0=ALU.mult, op1=ALU.add)
                if with_ones:
                    nc.vector.memset(vTr[:, :, C:C + 1], 1.0)

                for ci, (cs, cw) in enumerate(CHUNKS):
                    av = psC.tile([128, 512], F32, tag="av")
                    if not with_ones:
                        zz = psB.tile([1, 512], F32, tag="psb")
                    for j in range(JT):
                        kt = (kown[:, 128 * j:128 * (j + 1)] if j < JL
                              else krem[:, 128 * (j - JL):128 * (j - JL + 1)])
                        vt = (vT[:, j, :] if j < JL else vTr[:, j - JL, :])
                        sc = psA.tile([128, 512], F32, tag="ps")
                        nc.tensor.matmul(sc[:, 0:cw], kt, q[:, cs:cs + cw],
                                         start=True, stop=True)
                        nc.scalar.activation(probs[:, j, 0:cw], sc[:, 0:cw], AF.Exp)
                        nc.tensor.matmul(av[0:vcols, 0:cw], vt, probs[:, j, 0:cw],
                                         start=(j == 0), stop=(j == JT - 1))
                        if not with_ones:
                            nc.tensor.matmul(zz[:, 0:cw], ones[:], probs[:, j, 0:cw],
                                             start=(j == 0), stop=(j == JT - 1))
                    zrow = av[C:C + 1, 0:cw] if with_ones else zz[:, 0:cw]
                    rz = sp.tile([1, 512], F32, tag="rz")
                    nc.vector.reciprocal(rz[:, 0:cw], zrow)
                    nc.vector.scalar_tensor_tensor(
                        out=rz[:, 0:cw], in0=rz[:, 0:cw], scalar=gamma,
                        in1=mask[:, cs:cs + cw], op0=ALU.mult, op1=ALU.mult)
                    rzb = sp.tile([C, 512], F32, tag="rzb")
                    nc.gpsimd.partition_broadcast(rzb[:, 0:cw], rz[:, 0:cw])
                    tmp = sp.tile([C, 512], F32, tag="at_avtmp")
                    nc.vector.tensor_mul(tmp[:, 0:cw], av[0:C, 0:cw],
                                         rzb[:, 0:cw])
                    nc.vector.tensor_add(
                        xa_out[xa_po:xa_po + C, xa_xo + cs:xa_xo + cs + cw],
                        tmp[:, 0:cw], x[0:C, cs:cs + cw])

            probs1 = pp.tile([128, JT, 512], BF16, tag="probs")
            x1ad = ap_.tile([128, WP], BF16, tag="x1ad")
            nc.vector.memset(x1ad[:, 0:PD], 0.0)
            nc.vector.memset(x1ad[:, WP - PD:WP], 0.0)
            attention(x1, 64, 8, wq1, wk1, wv1, gamma1, ag1_in, ag1_out,
                      65, probs1, x1ad, 0, PD, "1")
            # duplicate rows 64:128 = rows 0:64 shifted by +1 (for tap pairs)
            nc.sync.dma_start(x1ad[64:128, 0:WP - 1], x1ad[0:64, 1:WP])
            nc.vector.memset(x1ad[64:128, WP - 1:WP], 0.0)

            # ---------- conv2 (64 -> 128, 8 tap-pair matmuls) ----------
            st2 = sp.tile([128, 16], F32, tag="st2")
            y2 = ap_.tile([128, W], BF16, tag="y")
            sq2 = sp.tile([128, 600], F32, tag="sq")
            for ci, (cs, cw) in enumerate(CHUNKS):
                ps = psA.tile([128, 512], F32, tag="ps")
                for t in range(8):
                    nc.tensor.matmul(ps[:, 0:cw], w2p[:, t, :],
                                     x1ad[:, PD + cs + 2 * t - 7:PD + cs + 2 * t - 7 + cw],
                                     start=(t == 0), stop=(t == 7))
                a, wd = STAT_SL[ci]
                sl = ps[:, a - cs:a - cs + wd]
                nc.vector.tensor_reduce(st2[:, ci:ci + 1], sl,
                                        axis=mybir.AxisListType.X, op=ALU.add)
                nc.scalar.activation(sq2[:, 0:wd], sl, AF.Square,
                                     accum_out=st2[:, 5 + ci:6 + ci])
                nc.vector.tensor_copy(y2[:, cs:cs + cw], ps[:, 0:cw])
            stats2 = sp.tile([128, 2], F32, tag="stats2")
            nc.vector.tensor_reduce(stats2[:, 0:1], st2[:, 0:5],
                                    axis=mybir.AxisListType.X, op=ALU.add)
            nc.vector.tensor_reduce(stats2[:, 1:2], st2[:, 5:10],
                                    axis=mybir.AxisListType.X, op=ALU.add)
            nc.sync.dma_start(ar_in[1].ap(), stats2[:])
            nc.gpsimd.collective_compute(
                "AllGather", ALU.bypass, replica_groups=ALL8,
                ins=[ar_in[1].ap().opt()], outs=[ar_out[1].ap().opt()])
            ss2 = bn_scale_shift(ar_out[1], bn2g, bn2b, 128, "ss2")
            x2 = ap_.tile([128, WP], BF16, tag="x2")
            nc.vector.memset(x2[:, 0:PD], 0.0)
            nc.vector.memset(x2[:, WP - PD:WP], 0.0)
            bn_apply_mask(y2, x2, ss2, 128, xo=PD)

            # ---------- attention 2 (C=128, d=16) ----------
            probs2 = pp.tile([128, JT, 512], BF16, tag="probs")
            x2a = ap_.tile([128, WP], BF16, tag="x2a")
            nc.vector.memset(x2a[:, 0:PD], 0.0)
            nc.vector.memset(x2a[:, WP - PD:WP], 0.0)
            x2v = x2[:, PD:PD + W]
            attention(x2v, 128, 16, wq2, wk2, wv2, gamma2, ag2_in, ag2_out,
                      128, probs2, x2a, 0, PD, "2")

            # ---------- conv3 (128 -> 64, 15 taps) ----------
            st3 = sp.tile([64, 16], F32, tag="st3")
            y3 = ap_.tile([64, W], BF16, tag="y")
            sq3 = sp.tile([64, 600], F32, tag="sq")
            for ci, (cs, cw) in enumerate(CHUNKS):
                ps = psA.tile([64, 512], F32, tag="ps")
                for t in range(15):
                    nc.tensor.matmul(ps[0:64, 0:cw], w3t[:, t, :],
                                     x2a[:, PD + cs + t - 7:PD + cs + t - 7 + cw],
                                     start=(t == 0), stop=(t == 14))
                a, wd = STAT_SL[ci]
                sl = ps[0:64, a - cs:a - cs + wd]
                nc.vector.tensor_reduce(st3[:, ci:ci + 1], sl,
                                        axis=mybir.AxisListType.X, op=ALU.add)
                nc.scalar.activation(sq3[:, 0:wd], sl, AF.Square,
                                     accum_out=st3[:, 5 + ci:6 + ci])
                nc.vector.tensor_copy(y3[:, cs:cs + cw], ps[0:64, 0:cw])
            stats3 = sp.tile([64, 2], F32, tag="stats3")
            nc.vector.tensor_reduce(stats3[:, 0:1], st3[:, 0:5],
                                    axis=mybir.AxisListType.X, op=ALU.add)
            nc.vector.tensor_reduce(stats3[:, 1:2], st3[:, 5:10],
                                    axis=mybir.AxisListType.X, op=ALU.add)
            nc.sync.dma_start(ar_in[2][0:64, :], stats3[:])
            nc.gpsimd.collective_compute(
                "AllGather", ALU.bypass, replica_groups=ALL8,
                ins=[ar_in[2].ap().opt()], outs=[ar_out[2].ap().opt()])
            ss3 = bn_scale_shift(ar_out[2], bn3g, bn3b, 64, "ss3")
            x3d = ap_.tile([128, WP], BF16, tag="x3d")
            nc.vector.memset(x3d[:, 0:PD], 0.0)
            nc.vector.memset(x3d[:, WP - PD:WP], 0.0)
            bn_apply_mask(y3, x3d, ss3, 64, xo=PD)
            nc.sync.dma_start(x3d[64:128, 0:WP - 1], x3d[0:64, 1:WP])
            nc.vector.memset(x3d[64:128, WP - 1:WP], 0.0)

            # ---------- conv4 (64 -> 32, 8 tap-pairs) + relu ----------
            x4q = ap_.tile([128, WP], BF16, tag="x4q")
            nc.vector.memset(x4q[:, 0:PD], 0.0)
            nc.vector.memset(x4q[:, WP - PD:WP], 0.0)
            for ci, (cs, cw) in enumerate(CHUNKS):
                ps = psA.tile([32, 512], F32, tag="ps")
                for t in range(8):
                    nc.tensor.matmul(ps[:, 0:cw], w4p[:, t, :],
                                     x3d[:, PD + cs + 2 * t - 7:PD + cs + 2 * t - 7 + cw],
                                     start=(t == 0), stop=(t == 7))
                nc.vector.tensor_scalar(x4q[0:32, PD + cs:PD + cs + cw],
                                        ps[:, 0:cw], scalar1=c4b[:], scalar2=0.0,
                                        op0=ALU.add, op1=ALU.max)
            nc.vector.tensor_mul(x4q[0:32, PD:PD + W], x4q[0:32, PD:PD + W],
                                 mask_b[0:32, :])
            for k in range(1, 4):
                nc.sync.dma_start(x4q[32 * k:32 * k + 32, 0:WP - k],
                                  x4q[0:32, k:WP])
                nc.vector.memset(x4q[32 * k:32 * k + 32, WP - k:WP], 0.0)

            # ---------- conv5 (32 -> 1, 4 tap-quad matmuls) + output ----------
            for ci, (cs, cw) in enumerate(OUT_CHUNKS):
                ps = psB.tile([1, 512], F32, tag="psb")
                for g in range(4):
                    nc.tensor.matmul(ps[:], w5g[:, g, :],
                                     x4q[:, PD + cs - 7 + 4 * g:PD + cs - 7 + 4 * g + cw],
                                     start=(g == 0), stop=(g == 3))
                oc = sp.tile([1, 512], F32, tag="oc")
                nc.vector.tensor_scalar_mul(oc[:], ps[:], STRENGTH)
                nc.vector.tensor_scalar_add(oc[:], oc[:], STRENGTH * c5b_f)
                nc.vector.tensor_add(oc[:], oc[:],
                                     aud[:, cs - MG:cs - MG + cw])
                nc.sync.dma_start(out_d[:, cs - MG:cs - MG + cw], oc[:])

    nc.compile()
    return nc


def kernel(audio, message, w1, c1b, w2, c2b, w3, c3b, w4, c4b, w5, c5b,
           bn1_g, bn1_b, bn2_g, bn2_b, bn3_g, bn3_b,
           a1_wq, a1_wk, a1_wv, a1_g, a2_wq, a2_wk, a2_wv, a2_g,
           _trace=False):
    global LAST_RESULTS
    audio = np.asarray(audio); message = np.asarray(message)

    # conv biases c1b/c2b/c3b cancel exactly inside training-mode BatchNorm
    # (BN(x + const) == BN(x)); c4b/c5b are applied on device / baked.
    w1 = np.asarray(w1); w2 = np.asarray(w2); w3 = np.asarray(w3)
    w4 = np.asarray(w4); w5 = np.asarray(w5)

    w1c = np.zeros((30, 64), np.float32)
    for t in range(15):
        for ch in range(2):
            w1c[2 * t + ch, :] = w1[:, ch, t]
    w2p = np.zeros((8, 128, 128), np.float32)
    for t in range(8):
        w2p[t, 0:64, :] = w2[:, :, 2 * t].T
        if 2 * t + 1 < 15:
            w2p[t, 64:128, :] = w2[:, :, 2 * t + 1].T
    w3t = np.zeros((15, 128, 64), np.float32)
    for t in range(15):
        w3t[t] = w3[:, :, t].T
    w4p = np.zeros((8, 128, 32), np.float32)
    for t in range(8):
        w4p[t, 0:64, :] = w4[:, :, 2 * t].T
        if 2 * t + 1 < 15:
            w4p[t, 64:128, :] = w4[:, :, 2 * t + 1].T
    w5g = np.zeros((4, 128, 1), np.float32)
    for g in range(4):
        for k in range(4):
            t = 4 * g + k
            if t < 15:
                w5g[g, 32 * k:32 * k + 32, 0] = w5[0, :, t]

    common = {
        "w1c": _f32(w1c),
        "wq1": _bf(np.asarray(a1_wq).T), "wk1": _bf(np.asarray(a1_wk).T),
        "wv1": _bf(np.asarray(a1_wv).T),
        "w2p": _bf(np.transpose(w2p, (1, 0, 2))),
        "wq2": _bf(np.asarray(a2_wq).T), "wk2": _bf(np.asarray(a2_wk).T),
        "wv2": _bf(np.asarray(a2_wv).T),
        "w3t": _bf(np.transpose(w3t, (1, 0, 2))),
        "w4p": _bf(np.transpose(w4p, (1, 0, 2))),
        "w5g": _bf(np.transpose(w5g, (1, 0, 2))),
        "bn1g": _f32(np.asarray(bn1_g).reshape(64, 1)),
        "bn1b": _f32(np.asarray(bn1_b).reshape(64, 1)),
        "bn2g": _f32(np.asarray(bn2_g).reshape(128, 1)),
        "bn2b": _f32(np.asarray(bn2_b).reshape(128, 1)),
        "bn3g": _f32(np.asarray(bn3_g).reshape(64, 1)),
        "bn3b": _f32(np.asarray(bn3_b).reshape(64, 1)),
        "c4b": _f32(np.asarray(c4b).reshape(32, 1)),
    }

    in_maps = []
    for core in range(NCORES):
        b, h = core // 2, core % 2
        s0 = h * OWN
        x0 = np.zeros((2, WIN), np.float32)
        lo, hi = s0 - INM, s0 + OWN + INM
        slo, shi = max(lo, 0), min(hi, L)
        x0[0, slo - lo:shi - lo] = audio[b, 0, slo:shi]
        x0[1, slo - lo:shi - lo] = message[b, 0, slo:shi]
        mask = np.zeros((1, W), np.float32)
        g = np.arange(s0 - MG, s0 + OWN + MG)
        mask[0] = ((g >= 0) & (g < L)).astype(np.float32)
        m01 = np.zeros((128, 2), np.float32)
        m01[:, 0] = 1.0 - h
        m01[:, 1] = h
        im = dict(common)
        im.update({
            "x0": x0,
            "aud": _f32(audio[b, :, s0:s0 + OWN]),
            "mask": mask,
            "m01": m01,
        })
        in_maps.append(im)

    nc = build_graph(float(np.asarray(a1_g)), float(np.asarray(a2_g)),
                     float(np.asarray(c5b).reshape(-1)[0]))
    res = run_bass_kernel_spmd(nc, in_maps, core_ids=list(range(NCORES)),
                               trace=_trace)
    LAST_RESULTS = res

    out = np.zeros((B, 1, L), np.float32)
    for core in range(NCORES):
        b, h = core // 2, core % 2
        out[b, 0, h * OWN:(h + 1) * OWN] = res.results[core]["out"][0]
    return out

